# revision 15
# baseline (speedup 1.0000x reference)
"""Trainium2 Bass kernel for nn_Decoder_TNTM (topic-model decoder).

out[b,v] = logsumexp_k( log_beta[k,v] + log_softmax(theta_hat)[b,k] )

Math (validated against the jax reference):
  With Woodbury, Sigma_k^{-1} = Dinv - U_k U_k^T  (U = Dinv W Lc^{-T}),
  log_beta[k,v] = A_k + e_v.b_k + (e_v^2).c_k + 0.5||U_k^T e_v||^2.
  K-sized coefficients (A, b, c, U, theta) are computed on the host in
  float64. A_k is folded into theta: theta'[k,b] = softmax[b,k]*exp(A_k -
  maxA), compensated by +maxA on the output (exact: the per-column max m
  cancels between the exp bias and the final fixup).

Device work per core (V sharded 8 ways, 6272 rows in 49 tiles of 128):
  PE : Z = E @ (16*U) as fp8 e4m3 DoubleRow matmuls (d=256 contraction in
       one instruction); G = E@b + E^2@c in bf16 (exact vs the fp8-rounded
       E); transpose of the exp-row; final theta matmul in bf16.
  ACT: Square((1/16) Z) PSUM->SBUF bf16 (most chunks), exp, ln.
  DVE: one square chunk on some tiles, one L1 add, the segmented reduce
       (only DVE can reduce along the free axis), logb, max, fixups.
  Pool(gpsimd): three of the four L1 pairwise adds (bf16, SBUF only).
"""

import numpy as np
from contextlib import ExitStack

import bass_rust
import concourse.bass as bass
import concourse.mybir as mybir
import concourse.tile as tile
from concourse.bass_utils import run_bass_kernel_spmd
from concourse.masks import make_identity
from concourse.vector_clock import ScopedClock, VectorClock

F32 = mybir.dt.float32
BF16 = mybir.dt.bfloat16
FP8 = mybir.dt.float8e4
AF = mybir.ActivationFunctionType
ALU = mybir.AluOpType
DR = mybir.MatmulPerfMode.DoubleRow

N_CORES = 8
B, K, V, D, R = 64, 64, 50000, 256, 64
KR = K * R                       # 4096
V_PER_CORE = V // N_CORES        # 6250
N_VT = (V_PER_CORE + 127) // 128  # 49
V_PAD = N_VT * 128               # 6272
LOG_2PI = float(np.log(2.0 * np.pi))
USCALE = 16.0                    # U is uploaded *16 (fp8 range), squares /256


class _SplitDrainTileContext(tile.TileContext):
    """This container's walrus rejects >1 sem wait per CTRL-class
    instruction; split the tail drain's waits across sync NOPs."""

    def _drain_and_barrier(self, tick_clock, wait_clock):
        gc = tick_clock.global_clock
        nproc = len(gc)
        for p in (i for i in range(nproc) if gc[i] > 0):
            vec = [0] * nproc
            vec[p] = gc[p]
            nop_inst = self.nc.sync.nop(nofuse=True)
            wait_clock.add_sem_waits(
                nop_inst.ins, ScopedClock({None: VectorClock(vec)})
            )
        self.nc.sync.drain()

        self.nc.all_engine_barrier()
        assert self.sems is not None
        popped = self.nc._tile_sem_poison_stack.pop()
        assert popped is self._sem_poison
        self.nc.clear_and_free_semaphores(list(self.sems.allocated().values()))
        self.nc.all_engine_barrier()


def _split_multi_waits(nc, max_waits=1):
    """Walrus here rejects instructions carrying more than one sem wait.
    Hoist excess waits onto same-engine NOPs inserted just before the
    offending instruction (same program point, so semantics unchanged)."""
    ctr = 0
    nsplit = 0
    for fn in nc.m.functions:
        for bb in fn.blocks:
            il = bb.instructions
            out = []
            changed = False
            for inst in il:
                si = inst.sync_info
                waits = list(si.on_wait) if si is not None and si.on_wait else []
                if len(waits) > max_waits:
                    nsplit += 1
                    extra = waits[max_waits:]
                    for c0 in range(0, len(extra), max_waits):
                        nop = mybir.InstNoOp(
                            name=f"waitsplit_{ctr}", ins=[], outs=[])
                        ctr += 1
                        nop.engine = inst.engine
                        nop.sync_info = bass_rust.SyncInfo(
                            on_wait=extra[c0:c0 + max_waits], on_update=[])
                        out.append(nop)
                    si.on_wait = waits[:max_waits]
                    changed = True
                out.append(inst)
            if changed:
                il[:] = out
    return nsplit


AMAX = 0.0  # set per-build; compensation for the A-fold into theta


def emit(nc, tc, aps, n_vt=N_VT):
    ET8, ET16, E2T16, U8, B16, C16, thetaT, outT = (
        aps["ET8"], aps["ET16"], aps["E2T16"], aps["U8"], aps["B16"],
        aps["C16"], aps["thetaT"], aps["outT"],
    )
    with ExitStack() as ctx:
        cst = ctx.enter_context(tc.tile_pool(name="cst", bufs=1))

        u8 = cst.tile([128, 2 * KR], FP8)
        nc.sync.dma_start(u8[:], U8[:])
        b16 = cst.tile([128, 2 * K], BF16)
        nc.sync.dma_start(b16[:], B16[:])
        c16 = cst.tile([128, 2 * K], BF16)
        nc.sync.dma_start(c16[:], C16[:])
        tht = cst.tile([K, B], BF16)
        nc.sync.dma_start(tht[:], thetaT[:])
        ident = cst.tile([128, 128], F32)
        make_identity(nc, ident[:])

        u8v = u8[:].rearrange("p (i j) -> p i j", i=2)

        etp = ctx.enter_context(tc.tile_pool(name="etp", bufs=4))
        y1p = ctx.enter_context(tc.tile_pool(name="y1p", bufs=3))
        z2p = ctx.enter_context(tc.tile_pool(name="z2p", bufs=4))
        zbp = ctx.enter_context(tc.tile_pool(name="zbp", bufs=2))
        smp = ctx.enter_context(tc.tile_pool(name="smp", bufs=4))
        outp = ctx.enter_context(tc.tile_pool(name="outp", bufs=4))

        zps = ctx.enter_context(tc.tile_pool(name="zps", bufs=2, space="PSUM"))
        gps = ctx.enter_context(tc.tile_pool(name="gps", bufs=4, space="PSUM"))

        def stage1(vt):
            """Z matmuls, squares to bf16 z2, L1 adds, segmented reduce."""
            sl = slice(vt * 128, (vt + 1) * 128)
            et8 = etp.tile([128, 256], FP8, tag="et8", name=f"et8_{vt}")
            nc.sync.dma_start(
                et8[:].rearrange("p (i v) -> p i v", i=2),
                ET8[:].rearrange("p (i v) -> p i v", i=2)[:, :, sl])
            et16 = etp.tile([128, 256], BF16, tag="et16", name=f"et16_{vt}")
            nc.sync.dma_start(
                et16[:].rearrange("p (i v) -> p i v", i=2),
                ET16[:].rearrange("p (i v) -> p i v", i=2)[:, :, sl])
            e2t16 = etp.tile([128, 256], BF16, tag="e2t16",
                             name=f"e2t16_{vt}")
            nc.sync.dma_start(
                e2t16[:].rearrange("p (i v) -> p i v", i=2),
                E2T16[:].rearrange("p (i v) -> p i v", i=2)[:, :, sl])
            et8_ap = et8[:].rearrange("p (i v) -> p i v", i=2)

            # one chunk on DVE every 4th tile rebalances ACT (measured).
            n_act = 4 if vt % 4 < 3 else 3

            y1 = y1p.tile([128, 2048], BF16, tag="y1", name=f"y1_{vt}")
            for c in range(4):
                zt = zps.tile([128, 1024], F32, tag="zp", name=f"zp_{vt}_{c}")
                for h in range(2):
                    j0 = c * 1024 + h * 512
                    nc.tensor.matmul(
                        zt[:, h * 512:(h + 1) * 512],
                        et8_ap,
                        u8v[:, :, j0:j0 + 512],
                        start=True, stop=True, perf_mode=DR,
                    )
                z2c = z2p.tile([128, 1024], BF16, tag="z2",
                               name=f"z2_{vt}_{c}")
                if c < n_act:
                    nc.scalar.activation(z2c[:], zt[:], AF.Square,
                                         scale=1.0 / USCALE)
                else:
                    zb = zbp.tile([128, 1024], BF16, tag="zb",
                                  name=f"zb_{vt}_{c}")
                    nc.vector.tensor_scalar(
                        zb[:], zt[:], 1.0 / USCALE, None, op0=ALU.mult)
                    nc.vector.tensor_tensor(z2c[:], zb[:], zb[:],
                                            op=ALU.mult)
                # L1: z2 [p,16,64] -> y1 chunk [p,16,32]
                z2seg = z2c[:].rearrange("p (s r) -> p s r", r=64)
                y1c = y1[:, c * 512:(c + 1) * 512].rearrange(
                    "p (s r) -> p s r", r=32)
                l1_eng = nc.vector if c == 3 else nc.gpsimd
                l1_eng.tensor_tensor(
                    y1c, z2seg[:, :, 0:32], z2seg[:, :, 32:64], op=ALU.add)

            # segmented reduce y1 [p,64,32] -> s2 [p,64] (DVE only)
            s2 = smp.tile([128, K], F32, tag="s2", name=f"s2_{vt}")
            nc.vector.tensor_reduce(
                s2[:], y1[:].rearrange("p (s r) -> p s r", r=32),
                axis=mybir.AxisListType.X, op=ALU.add,
            )
            return et16, e2t16, s2

        def stage1b(vt, et16, e2t16, s2):
            """G = E.b + E^2.c (bf16 matmuls), logb = s2 + G, mneg.
            Deferred one tile so the PE never waits on this tile's tree."""
            g = gps.tile([128, 128], F32, tag="gs", name=f"g_{vt}")
            nc.tensor.matmul(g[:, :K], et16[:, 0:128], b16[:, 0:K],
                             start=True, stop=False)
            nc.tensor.matmul(g[:, :K], et16[:, 128:256], b16[:, K:2 * K],
                             start=False, stop=False)
            nc.tensor.matmul(g[:, :K], e2t16[:, 0:128], c16[:, 0:K],
                             start=False, stop=False)
            nc.tensor.matmul(g[:, :K], e2t16[:, 128:256], c16[:, K:2 * K],
                             start=False, stop=True)
            logb = smp.tile([128, K], F32, tag="logb", name=f"logb_{vt}")
            nc.vector.tensor_tensor(logb[:], s2[:], g[:, :K], op=ALU.add)
            mneg = smp.tile([128, 1], F32, tag="mneg", name=f"mneg_{vt}")
            nc.vector.tensor_reduce(
                mneg[:], logb[:], axis=mybir.AxisListType.X, op=ALU.max,
                negate=True,
            )
            return logb, mneg

        def stage2(vt, logb, mneg):
            eb = smp.tile([128, K], F32, tag="eb", name=f"eb_{vt}")
            nc.scalar.activation(eb[:], logb[:], AF.Exp, bias=mneg[:],
                                 scale=1.0)

            x = gps.tile([128, 128], F32, tag="gs", name=f"x_{vt}")
            nc.tensor.transpose(x[:K, :], eb[:], ident[:])
            ebt = smp.tile([K, 128], BF16, tag="ebt", name=f"ebt_{vt}")
            nc.vector.tensor_copy(ebt[:], x[:K, :])
            nc.tensor.matmul(x[:, :B], ebt[:], tht[:], start=True, stop=True)

            # out = ln(S) + m + maxA
            outl = outp.tile([128, B], F32, tag="outl", name=f"outl_{vt}")
            nc.scalar.activation(outl[:], x[:, :B], AF.Ln)
            outr = outp.tile([128, B], F32, tag="outr", name=f"outr_{vt}")
            nc.vector.tensor_scalar(
                outr[:], outl[:], mneg[:], AMAX,
                op0=ALU.subtract, op1=ALU.add,
            )
            nc.sync.dma_start(outT[vt * 128:(vt + 1) * 128, :], outr[:])

        p1 = []
        p1b = []
        for vt in range(n_vt):
            p1.append(stage1(vt))
            if vt >= 1:
                p1b.append(stage1b(vt - 1, *p1[vt - 1]))
            if vt >= 3:
                stage2(vt - 3, *p1b[vt - 3])
        p1b.append(stage1b(n_vt - 1, *p1[n_vt - 1]))
        for vt in range(n_vt - 3, n_vt):
            stage2(vt, *p1b[vt])


def build_program(n_vt=N_VT, split_waits=True, amax=0.0):
    global AMAX
    AMAX = float(amax)
    nc = bass.Bass("TRN2", target_bir_lowering=False, debug=False)
    aps = {
        "ET8": nc.dram_tensor(
            "ET8", [128, 2 * V_PAD], FP8, kind="ExternalInput").ap(),
        "ET16": nc.dram_tensor(
            "ET16", [128, 2 * V_PAD], BF16, kind="ExternalInput").ap(),
        "E2T16": nc.dram_tensor(
            "E2T16", [128, 2 * V_PAD], BF16, kind="ExternalInput").ap(),
        "U8": nc.dram_tensor(
            "U8", [128, 2 * KR], FP8, kind="ExternalInput").ap(),
        "B16": nc.dram_tensor(
            "B16", [128, 2 * K], BF16, kind="ExternalInput").ap(),
        "C16": nc.dram_tensor(
            "C16", [128, 2 * K], BF16, kind="ExternalInput").ap(),
        "thetaT": nc.dram_tensor(
            "thetaT", [K, B], BF16, kind="ExternalInput").ap(),
        "outT": nc.dram_tensor(
            "outT", [V_PAD, B], F32, kind="ExternalOutput").ap(),
    }
    with _SplitDrainTileContext(nc) as tc:
        emit(nc, tc, aps, n_vt=n_vt)
    if split_waits:
        _split_multi_waits(nc)
    return nc


def _pack2(a, np_dt):
    """[D, N] f32 -> [128, 2N] with a2[p, i*N+n] = a[i*128+p, n]."""
    d, n = a.shape
    assert d == 256
    return np.ascontiguousarray(
        a.reshape(2, 128, n).transpose(1, 0, 2).reshape(128, 2 * n)
    ).astype(np_dt)


def host_precompute(theta_hat, mus, L_lower, log_diag):
    """All K-sized coefficients, in float64. Returns device arrays plus
    the amax compensation scalar."""
    th = np.asarray(theta_hat).astype(np.float64)
    mus = np.asarray(mus).astype(np.float64)
    L = np.asarray(L_lower).astype(np.float64)
    ld = np.asarray(log_diag).astype(np.float64)
    Kk, d, r = L.shape

    Dinv = np.exp(-ld)                                   # (K,d)
    Wd = L * Dinv[:, :, None]                            # (K,d,r)
    C = np.eye(r)[None] + np.einsum("kdr,kds->krs", L, Wd)
    Lc = np.linalg.cholesky(C)
    logdet = ld.sum(-1) + 2.0 * np.log(
        np.diagonal(Lc, axis1=-2, axis2=-1)).sum(-1)
    Lc_inv = np.linalg.inv(Lc)                           # (K,r,r)
    U = np.einsum("kdr,ksr->kds", Wd, Lc_inv)            # Wd @ Lc^{-T}
    alpha = np.einsum("kdr,kd->kr", U, mus)
    bcoef = Dinv * mus - np.einsum("kdr,kr->kd", U, alpha)
    ccoef = -0.5 * Dinv
    A = (-0.5 * (d * LOG_2PI + logdet
                 + np.einsum("kd,kd->k", Dinv * mus, mus))
         + 0.5 * (alpha ** 2).sum(-1))
    Us = U / np.sqrt(2.0) * USCALE                       # (K,d,r)

    amax = float(A.max())
    theta = np.exp(th - th.max(-1, keepdims=True))
    theta /= theta.sum(-1, keepdims=True)                # (B,K)
    thetaT = theta.T * np.exp(A - amax)[:, None]         # (K,B)

    fp8 = mybir.dt.np(FP8)
    bf16 = mybir.dt.np(BF16)
    return {
        "U8": _pack2(Us.transpose(1, 0, 2).reshape(d, Kk * r), fp8),
        "B16": _pack2(bcoef.T, bf16),
        "C16": _pack2(ccoef.T, bf16),
        "thetaT": np.ascontiguousarray(thetaT).astype(bf16),
    }, amax


def make_in_maps(embeddings, pre):
    emb = np.asarray(embeddings, dtype=np.float32)
    fp8 = mybir.dt.np(FP8)
    bf16 = mybir.dt.np(BF16)
    in_maps = []
    for c in range(N_CORES):
        esl = np.zeros((V_PAD, D), np.float32)
        esl[:V_PER_CORE] = emb[c * V_PER_CORE:(c + 1) * V_PER_CORE]
        e8 = esl.astype(fp8)                             # round once
        e8f = e8.astype(np.float32)

        # [V_PAD, 256] -> [V_PAD, 2, 128] -> [128, 2, V_PAD] -> [128, 2V]
        def pack(a, np_dt):
            return np.ascontiguousarray(
                a.reshape(V_PAD, 2, 128).transpose(2, 1, 0).reshape(
                    128, 2 * V_PAD)).astype(np_dt)
        in_maps.append({
            "ET8": pack(e8f, fp8),
            "ET16": pack(esl, bf16),
            "E2T16": pack(esl * esl, bf16),
            **pre,
        })
    return in_maps


_NC_CACHE = None
_NC_CACHE_AMAX = None


def kernel(theta_hat, embeddings, mus, L_lower, log_diag):
    global _NC_CACHE, _NC_CACHE_AMAX
    pre, amax = host_precompute(theta_hat, mus, L_lower, log_diag)
    if _NC_CACHE is None or _NC_CACHE_AMAX != amax:
        _NC_CACHE = build_program(amax=amax)
        _NC_CACHE_AMAX = amax
    nc = _NC_CACHE
    in_maps = make_in_maps(embeddings, pre)
    res = run_bass_kernel_spmd(nc, in_maps, list(range(N_CORES)))
    out = np.empty((B, V), np.float32)
    for c in range(N_CORES):
        out[:, c * V_PER_CORE:(c + 1) * V_PER_CORE] = \
            res.results[c]["outT"][:V_PER_CORE].T
    return out


# revision 16
# speedup vs baseline: 1.0568x; 1.0568x over previous
"""Trainium2 Bass kernel for nn_Decoder_TNTM (topic-model decoder).

out[b,v] = logsumexp_k( log_beta[k,v] + log_softmax(theta_hat)[b,k] )

Math (validated against the jax reference):
  With Woodbury, Sigma_k^{-1} = Dinv - U_k U_k^T  (U = Dinv W Lc^{-T}),
  log_beta[k,v] = A_k + e_v.b_k + (e_v^2).c_k + 0.5||U_k^T e_v||^2.
  K-sized coefficients (A, b, c, U, theta) are computed on the host in
  float64. A_k is folded into theta: theta'[k,b] = softmax[b,k]*exp(A_k -
  maxA), compensated by +maxA on the output (exact: the per-column max m
  cancels between the exp bias and the final fixup).

Device work per core (V sharded 8 ways, 6272 rows in 49 tiles of 128):
  PE : Z = E @ (16*U) as fp8 e4m3 DoubleRow matmuls (d=256 contraction in
       one instruction); G = E@b + E^2@c in bf16 (exact vs the fp8-rounded
       E); transpose of the exp-row; final theta matmul in bf16.
  ACT: Square((1/16) Z) PSUM->SBUF bf16 (most chunks), exp, ln.
  DVE: one square chunk on some tiles, one L1 add, the segmented reduce
       (only DVE can reduce along the free axis), logb, max, fixups.
  Pool(gpsimd): three of the four L1 pairwise adds (bf16, SBUF only).
"""

import numpy as np
from contextlib import ExitStack

import bass_rust
import concourse.bass as bass
import concourse.mybir as mybir
import concourse.tile as tile
from concourse.bass_utils import run_bass_kernel_spmd
from concourse.masks import make_identity
from concourse.vector_clock import ScopedClock, VectorClock

F32 = mybir.dt.float32
BF16 = mybir.dt.bfloat16
FP8 = mybir.dt.float8e4
AF = mybir.ActivationFunctionType
ALU = mybir.AluOpType
DR = mybir.MatmulPerfMode.DoubleRow

N_CORES = 8
B, K, V, D, R = 64, 64, 50000, 256, 64
KR = K * R                       # 4096
V_PER_CORE = V // N_CORES        # 6250
N_VT = (V_PER_CORE + 127) // 128  # 49
V_PAD = N_VT * 128               # 6272
LOG_2PI = float(np.log(2.0 * np.pi))
USCALE = 16.0                    # U is uploaded *16 (fp8 range), squares /256
L1_ALL_DVE = True                # debug: bypass Pool for the L1 adds


class _SplitDrainTileContext(tile.TileContext):
    """This container's walrus rejects >1 sem wait per CTRL-class
    instruction; split the tail drain's waits across sync NOPs."""

    def _drain_and_barrier(self, tick_clock, wait_clock):
        gc = tick_clock.global_clock
        nproc = len(gc)
        for p in (i for i in range(nproc) if gc[i] > 0):
            vec = [0] * nproc
            vec[p] = gc[p]
            nop_inst = self.nc.sync.nop(nofuse=True)
            wait_clock.add_sem_waits(
                nop_inst.ins, ScopedClock({None: VectorClock(vec)})
            )
        self.nc.sync.drain()

        self.nc.all_engine_barrier()
        assert self.sems is not None
        popped = self.nc._tile_sem_poison_stack.pop()
        assert popped is self._sem_poison
        self.nc.clear_and_free_semaphores(list(self.sems.allocated().values()))
        self.nc.all_engine_barrier()


def _split_multi_waits(nc, max_waits=1):
    """Walrus here rejects instructions carrying more than one sem wait.
    Hoist excess waits onto same-engine NOPs inserted just before the
    offending instruction (same program point, so semantics unchanged)."""
    ctr = 0
    nsplit = 0
    for fn in nc.m.functions:
        for bb in fn.blocks:
            il = bb.instructions
            out = []
            changed = False
            for inst in il:
                si = inst.sync_info
                waits = list(si.on_wait) if si is not None and si.on_wait else []
                if len(waits) > max_waits:
                    nsplit += 1
                    extra = waits[max_waits:]
                    for c0 in range(0, len(extra), max_waits):
                        nop = mybir.InstNoOp(
                            name=f"waitsplit_{ctr}", ins=[], outs=[])
                        ctr += 1
                        nop.engine = inst.engine
                        nop.sync_info = bass_rust.SyncInfo(
                            on_wait=extra[c0:c0 + max_waits], on_update=[])
                        out.append(nop)
                    si.on_wait = waits[:max_waits]
                    changed = True
                out.append(inst)
            if changed:
                il[:] = out
    return nsplit


AMAX = 0.0  # set per-build; compensation for the A-fold into theta


def emit(nc, tc, aps, n_vt=N_VT):
    ET8, ET16, E2T16, U8, B16, C16, thetaT, outT = (
        aps["ET8"], aps["ET16"], aps["E2T16"], aps["U8"], aps["B16"],
        aps["C16"], aps["thetaT"], aps["outT"],
    )
    with ExitStack() as ctx:
        cst = ctx.enter_context(tc.tile_pool(name="cst", bufs=1))

        u8 = cst.tile([128, 2 * KR], FP8)
        nc.sync.dma_start(u8[:], U8[:])
        b16 = cst.tile([128, 2 * K], BF16)
        nc.sync.dma_start(b16[:], B16[:])
        c16 = cst.tile([128, 2 * K], BF16)
        nc.sync.dma_start(c16[:], C16[:])
        tht = cst.tile([K, B], BF16)
        nc.sync.dma_start(tht[:], thetaT[:])
        ident = cst.tile([128, 128], F32)
        make_identity(nc, ident[:])

        u8v = u8[:].rearrange("p (i j) -> p i j", i=2)

        etp = ctx.enter_context(tc.tile_pool(name="etp", bufs=4))
        y1p = ctx.enter_context(tc.tile_pool(name="y1p", bufs=3))
        z2p = ctx.enter_context(tc.tile_pool(name="z2p", bufs=4))
        zbp = ctx.enter_context(tc.tile_pool(name="zbp", bufs=2))
        smp = ctx.enter_context(tc.tile_pool(name="smp", bufs=4))
        outp = ctx.enter_context(tc.tile_pool(name="outp", bufs=4))

        zps = ctx.enter_context(tc.tile_pool(name="zps", bufs=2, space="PSUM"))
        gps = ctx.enter_context(tc.tile_pool(name="gps", bufs=4, space="PSUM"))

        def stage1(vt):
            """Z matmuls, squares to bf16 z2, L1 adds, segmented reduce."""
            sl = slice(vt * 128, (vt + 1) * 128)
            et8 = etp.tile([128, 256], FP8, tag="et8", name=f"et8_{vt}")
            nc.sync.dma_start(
                et8[:].rearrange("p (i v) -> p i v", i=2),
                ET8[:].rearrange("p (i v) -> p i v", i=2)[:, :, sl])
            et16 = etp.tile([128, 256], BF16, tag="et16", name=f"et16_{vt}")
            nc.sync.dma_start(
                et16[:].rearrange("p (i v) -> p i v", i=2),
                ET16[:].rearrange("p (i v) -> p i v", i=2)[:, :, sl])
            e2t16 = etp.tile([128, 256], BF16, tag="e2t16",
                             name=f"e2t16_{vt}")
            nc.sync.dma_start(
                e2t16[:].rearrange("p (i v) -> p i v", i=2),
                E2T16[:].rearrange("p (i v) -> p i v", i=2)[:, :, sl])
            et8_ap = et8[:].rearrange("p (i v) -> p i v", i=2)

            # one chunk on DVE every 4th tile rebalances ACT (measured).
            n_act = 4 if vt % 4 < 3 else 3

            y1 = y1p.tile([128, 2048], BF16, tag="y1", name=f"y1_{vt}")
            for c in range(4):
                zt = zps.tile([128, 1024], F32, tag="zp", name=f"zp_{vt}_{c}")
                for h in range(2):
                    j0 = c * 1024 + h * 512
                    nc.tensor.matmul(
                        zt[:, h * 512:(h + 1) * 512],
                        et8_ap,
                        u8v[:, :, j0:j0 + 512],
                        start=True, stop=True, perf_mode=DR,
                    )
                z2c = z2p.tile([128, 1024], BF16, tag="z2",
                               name=f"z2_{vt}_{c}")
                if c < n_act:
                    nc.scalar.activation(z2c[:], zt[:], AF.Square,
                                         scale=1.0 / USCALE)
                else:
                    zb = zbp.tile([128, 1024], BF16, tag="zb",
                                  name=f"zb_{vt}_{c}")
                    nc.vector.tensor_scalar(
                        zb[:], zt[:], 1.0 / USCALE, None, op0=ALU.mult)
                    nc.vector.tensor_tensor(z2c[:], zb[:], zb[:],
                                            op=ALU.mult)
                # L1: z2 [p,16,64] -> y1 chunk [p,16,32]
                z2seg = z2c[:].rearrange("p (s r) -> p s r", r=64)
                y1c = y1[:, c * 512:(c + 1) * 512].rearrange(
                    "p (s r) -> p s r", r=32)
                l1_eng = nc.vector if L1_ALL_DVE or c == 3 else nc.gpsimd
                l1_eng.tensor_tensor(
                    y1c, z2seg[:, :, 0:32], z2seg[:, :, 32:64], op=ALU.add)

            # segmented reduce y1 [p,64,32] -> s2 [p,64] (DVE only)
            s2 = smp.tile([128, K], F32, tag="s2", name=f"s2_{vt}")
            nc.vector.tensor_reduce(
                s2[:], y1[:].rearrange("p (s r) -> p s r", r=32),
                axis=mybir.AxisListType.X, op=ALU.add,
            )
            return et16, e2t16, s2

        def stage1b(vt, et16, e2t16, s2):
            """G = E.b + E^2.c (bf16 matmuls), logb = s2 + G, mneg.
            Deferred one tile so the PE never waits on this tile's tree."""
            g = gps.tile([128, 128], F32, tag="gs", name=f"g_{vt}")
            nc.tensor.matmul(g[:, :K], et16[:, 0:128], b16[:, 0:K],
                             start=True, stop=False)
            nc.tensor.matmul(g[:, :K], et16[:, 128:256], b16[:, K:2 * K],
                             start=False, stop=False)
            nc.tensor.matmul(g[:, :K], e2t16[:, 0:128], c16[:, 0:K],
                             start=False, stop=False)
            nc.tensor.matmul(g[:, :K], e2t16[:, 128:256], c16[:, K:2 * K],
                             start=False, stop=True)
            logb = smp.tile([128, K], F32, tag="logb", name=f"logb_{vt}")
            nc.vector.tensor_tensor(logb[:], s2[:], g[:, :K], op=ALU.add)
            mneg = smp.tile([128, 1], F32, tag="mneg", name=f"mneg_{vt}")
            nc.vector.tensor_reduce(
                mneg[:], logb[:], axis=mybir.AxisListType.X, op=ALU.max,
                negate=True,
            )
            return logb, mneg

        def stage2(vt, logb, mneg):
            eb = smp.tile([128, K], F32, tag="eb", name=f"eb_{vt}")
            nc.scalar.activation(eb[:], logb[:], AF.Exp, bias=mneg[:],
                                 scale=1.0)

            x = gps.tile([128, 128], F32, tag="gs", name=f"x_{vt}")
            nc.tensor.transpose(x[:K, :], eb[:], ident[:])
            ebt = smp.tile([K, 128], BF16, tag="ebt", name=f"ebt_{vt}")
            nc.vector.tensor_copy(ebt[:], x[:K, :])
            nc.tensor.matmul(x[:, :B], ebt[:], tht[:], start=True, stop=True)

            # out = ln(S) + m + maxA
            outl = outp.tile([128, B], F32, tag="outl", name=f"outl_{vt}")
            nc.scalar.activation(outl[:], x[:, :B], AF.Ln)
            outr = outp.tile([128, B], F32, tag="outr", name=f"outr_{vt}")
            nc.vector.tensor_scalar(
                outr[:], outl[:], mneg[:], AMAX,
                op0=ALU.subtract, op1=ALU.add,
            )
            nc.sync.dma_start(outT[vt * 128:(vt + 1) * 128, :], outr[:])

        p1 = []
        p1b = []
        for vt in range(n_vt):
            p1.append(stage1(vt))
            if vt >= 1:
                p1b.append(stage1b(vt - 1, *p1[vt - 1]))
            if vt >= 3:
                stage2(vt - 3, *p1b[vt - 3])
        p1b.append(stage1b(n_vt - 1, *p1[n_vt - 1]))
        for vt in range(n_vt - 3, n_vt):
            stage2(vt, *p1b[vt])


def build_program(n_vt=N_VT, split_waits=True, amax=0.0):
    global AMAX
    AMAX = float(amax)
    nc = bass.Bass("TRN2", target_bir_lowering=False, debug=False)
    aps = {
        "ET8": nc.dram_tensor(
            "ET8", [128, 2 * V_PAD], FP8, kind="ExternalInput").ap(),
        "ET16": nc.dram_tensor(
            "ET16", [128, 2 * V_PAD], BF16, kind="ExternalInput").ap(),
        "E2T16": nc.dram_tensor(
            "E2T16", [128, 2 * V_PAD], BF16, kind="ExternalInput").ap(),
        "U8": nc.dram_tensor(
            "U8", [128, 2 * KR], FP8, kind="ExternalInput").ap(),
        "B16": nc.dram_tensor(
            "B16", [128, 2 * K], BF16, kind="ExternalInput").ap(),
        "C16": nc.dram_tensor(
            "C16", [128, 2 * K], BF16, kind="ExternalInput").ap(),
        "thetaT": nc.dram_tensor(
            "thetaT", [K, B], BF16, kind="ExternalInput").ap(),
        "outT": nc.dram_tensor(
            "outT", [V_PAD, B], F32, kind="ExternalOutput").ap(),
    }
    with _SplitDrainTileContext(nc) as tc:
        emit(nc, tc, aps, n_vt=n_vt)
    if split_waits:
        _split_multi_waits(nc)
    return nc


def _pack2(a, np_dt):
    """[D, N] f32 -> [128, 2N] with a2[p, i*N+n] = a[i*128+p, n]."""
    d, n = a.shape
    assert d == 256
    return np.ascontiguousarray(
        a.reshape(2, 128, n).transpose(1, 0, 2).reshape(128, 2 * n)
    ).astype(np_dt)


def host_precompute(theta_hat, mus, L_lower, log_diag):
    """All K-sized coefficients, in float64. Returns device arrays plus
    the amax compensation scalar."""
    th = np.asarray(theta_hat).astype(np.float64)
    mus = np.asarray(mus).astype(np.float64)
    L = np.asarray(L_lower).astype(np.float64)
    ld = np.asarray(log_diag).astype(np.float64)
    Kk, d, r = L.shape

    Dinv = np.exp(-ld)                                   # (K,d)
    Wd = L * Dinv[:, :, None]                            # (K,d,r)
    C = np.eye(r)[None] + np.einsum("kdr,kds->krs", L, Wd)
    Lc = np.linalg.cholesky(C)
    logdet = ld.sum(-1) + 2.0 * np.log(
        np.diagonal(Lc, axis1=-2, axis2=-1)).sum(-1)
    Lc_inv = np.linalg.inv(Lc)                           # (K,r,r)
    U = np.einsum("kdr,ksr->kds", Wd, Lc_inv)            # Wd @ Lc^{-T}
    alpha = np.einsum("kdr,kd->kr", U, mus)
    bcoef = Dinv * mus - np.einsum("kdr,kr->kd", U, alpha)
    ccoef = -0.5 * Dinv
    A = (-0.5 * (d * LOG_2PI + logdet
                 + np.einsum("kd,kd->k", Dinv * mus, mus))
         + 0.5 * (alpha ** 2).sum(-1))
    Us = U / np.sqrt(2.0) * USCALE                       # (K,d,r)

    amax = float(A.max())
    theta = np.exp(th - th.max(-1, keepdims=True))
    theta /= theta.sum(-1, keepdims=True)                # (B,K)
    thetaT = theta.T * np.exp(A - amax)[:, None]         # (K,B)

    fp8 = mybir.dt.np(FP8)
    bf16 = mybir.dt.np(BF16)
    return {
        "U8": _pack2(Us.transpose(1, 0, 2).reshape(d, Kk * r), fp8),
        "B16": _pack2(bcoef.T, bf16),
        "C16": _pack2(ccoef.T, bf16),
        "thetaT": np.ascontiguousarray(thetaT).astype(bf16),
    }, amax


def make_in_maps(embeddings, pre):
    emb = np.asarray(embeddings, dtype=np.float32)
    fp8 = mybir.dt.np(FP8)
    bf16 = mybir.dt.np(BF16)
    in_maps = []
    for c in range(N_CORES):
        esl = np.zeros((V_PAD, D), np.float32)
        esl[:V_PER_CORE] = emb[c * V_PER_CORE:(c + 1) * V_PER_CORE]
        e8 = esl.astype(fp8)                             # round once
        e8f = e8.astype(np.float32)

        # [V_PAD, 256] -> [V_PAD, 2, 128] -> [128, 2, V_PAD] -> [128, 2V]
        def pack(a, np_dt):
            return np.ascontiguousarray(
                a.reshape(V_PAD, 2, 128).transpose(2, 1, 0).reshape(
                    128, 2 * V_PAD)).astype(np_dt)
        in_maps.append({
            "ET8": pack(e8f, fp8),
            "ET16": pack(esl, bf16),
            "E2T16": pack(esl * esl, bf16),
            **pre,
        })
    return in_maps


_NC_CACHE = None
_NC_CACHE_AMAX = None


def kernel(theta_hat, embeddings, mus, L_lower, log_diag):
    global _NC_CACHE, _NC_CACHE_AMAX
    pre, amax = host_precompute(theta_hat, mus, L_lower, log_diag)
    if _NC_CACHE is None or _NC_CACHE_AMAX != amax:
        _NC_CACHE = build_program(amax=amax)
        _NC_CACHE_AMAX = amax
    nc = _NC_CACHE
    in_maps = make_in_maps(embeddings, pre)
    res = run_bass_kernel_spmd(nc, in_maps, list(range(N_CORES)))
    out = np.empty((B, V), np.float32)
    for c in range(N_CORES):
        out[:, c * V_PER_CORE:(c + 1) * V_PER_CORE] = \
            res.results[c]["outT"][:V_PER_CORE].T
    return out
